# revision 4
# baseline (speedup 1.0000x reference)
"""Trainium2 Bass kernel for nn_BTNetEuropean (binomial-tree European option pricer).

Reference computes x0 = relu(k @ w_init + b_init) then runs the linear
recurrence x <- w0*x + w1*shift(x) for N=1024 steps and returns x[:, 0].

Because the recurrence is linear with constant coefficients, the output is a
fixed linear functional of x0:

    out[b] = sum_j C(N,j) * w0^(N-j) * w1^j * relu(k[b]*w1row[j] + b_init[j])
           = sum_j ce_j * relu(k[b] + be_j)        (ce = c*w1row, be = b/w1row)

The coefficients c_j form a narrow binomial bump (sigma ~ 16 around j = N/2),
so only a small window of columns has a k-dependent relu sign for the actual
k range; every always-positive column folds into a per-batch FMA k*P + Q and
always-negative / negligible columns drop out. The device computes, per batch
element, relu over a W-column window, a weighted reduction, plus the FMA —
all derived on host in fp64 from the runtime inputs.

Sharding: pure data parallel over the batch of strikes across 8 NeuronCores.
"""

import math

import numpy as np

N_CORES = 8
BATCH = 8192
SHARD = BATCH // N_CORES  # 1024
P = 128  # SBUF partitions
GROUPS = SHARD // P  # 8 batch tiles of 128 per core

_COMPILED: dict[int, object] = {}


def _build_module(W: int):
    """Compile the SPMD Bass/Tile kernel for window width W (multiple of 128)."""
    import concourse.bacc as bacc
    import concourse.mybir as mybir
    import concourse.tile as tile

    f32 = mybir.dt.float32
    Alu = mybir.AluOpType
    Act = mybir.ActivationFunctionType

    nc = bacc.Bacc(
        "TRN2",
        debug=False,
        target_bir_lowering=False,
        num_devices=N_CORES,
    )
    kk_d = nc.dram_tensor("kk", [P, GROUPS], f32, kind="ExternalInput").ap()
    bw_d = nc.dram_tensor("bw", [P, W], f32, kind="ExternalInput").ap()
    cw_d = nc.dram_tensor("cw", [P, W], f32, kind="ExternalInput").ap()
    pq_d = nc.dram_tensor("pq", [P, 2], f32, kind="ExternalInput").ap()
    out_d = nc.dram_tensor("out", [P, GROUPS], f32, kind="ExternalOutput").ap()

    with tile.TileContext(nc) as tc:
        with (
            tc.tile_pool(name="const", bufs=1) as cpool,
            tc.tile_pool(name="work", bufs=2) as wpool,
        ):
            kk = cpool.tile([P, GROUPS], f32)
            nc.sync.dma_start(kk[:], kk_d[:])
            bw = cpool.tile([P, W], f32)
            nc.sync.dma_start(bw[:], bw_d[:])
            cw = cpool.tile([P, W], f32)
            nc.sync.dma_start(cw[:], cw_d[:])
            pq = cpool.tile([P, 2], f32)
            nc.sync.dma_start(pq[:], pq_d[:])

            # fma[p, g] = k[p, g] * P_fold + Q_fold
            fma = cpool.tile([P, GROUPS], f32)
            nc.vector.tensor_scalar(
                fma[:], kk[:], pq[:, 0:1], pq[:, 1:2], Alu.mult, Alu.add
            )

            # y_all[:, g*W:(g+1)*W] = relu(bw + k_g)  (scalar engine, per-partition bias)
            y_all = cpool.tile([P, GROUPS * W], f32)
            for g in range(GROUPS):
                nc.scalar.activation(
                    y_all[:, g * W : (g + 1) * W],
                    bw[:],
                    Act.Relu,
                    bias=kk[:, g : g + 1],
                )
            # prod = y_all * cw (cw broadcast across groups via stride-0 AP)
            prod = cpool.tile([P, GROUPS * W], f32)
            y3 = y_all[:].rearrange("p (g w) -> p g w", g=GROUPS)
            cw3 = cw[:].rearrange("p (g w) -> p g w", g=1).broadcast_to(
                [P, GROUPS, W]
            )
            nc.vector.tensor_tensor(
                prod[:].rearrange("p (g w) -> p g w", g=GROUPS), y3, cw3, Alu.mult
            )
            # reduce innermost axis: [P, G, W] -> [P, G], then add the FMA fold
            s = cpool.tile([P, GROUPS], f32)
            nc.vector.tensor_reduce(
                s[:], prod[:].rearrange("p (g w) -> p g w", g=GROUPS),
                axis=mybir.AxisListType.X, op=Alu.add,
            )
            res = cpool.tile([P, GROUPS], f32)
            nc.vector.tensor_add(res[:], s[:], fma[:])
            nc.sync.dma_start(out_d[:], res[:])
    nc.compile()
    return nc


def _get_module(W: int):
    if W not in _COMPILED:
        _COMPILED[W] = _build_module(W)
    return _COMPILED[W]


def kernel(k, w_init, b_init, w):
    k = np.asarray(k, dtype=np.float32)
    w_init = np.asarray(w_init, dtype=np.float32)
    b_init = np.asarray(b_init, dtype=np.float32)
    w = np.asarray(w, dtype=np.float32)

    n = b_init.shape[0] - 1  # 1024 recurrence steps
    assert k.shape == (BATCH, 1)

    # Host fp64 precompute: closed-form coefficients of the linear recurrence.
    j = np.arange(n + 1, dtype=np.float64)
    lg = math.lgamma
    logbinom = np.array(
        [lg(n + 1) - lg(jj + 1) - lg(n - jj + 1) for jj in j], dtype=np.float64
    )
    w64 = w.astype(np.float64)
    logc = logbinom + (n - j) * np.log(w64[0]) + j * np.log(w64[1])
    c = np.exp(logc)

    w1row = w_init[0].astype(np.float64)
    assert (w1row > 0).all(), "kernel assumes positive first-layer weights"
    ce = c * w1row  # effective weight per column
    be = b_init.astype(np.float64) / w1row  # effective bias per column

    kmin = float(k.min())
    kmax = float(k.max())
    neglig = ce < 1e-38  # below fp32 normal range; cannot move the output
    always_pos = (kmin + be >= 0.0) & ~neglig
    uncert = ~always_pos & (kmax + be > 0.0) & ~neglig

    p_fold = float(ce[always_pos].sum())
    q_fold = float((ce[always_pos] * be[always_pos]).sum())

    ui = np.where(uncert)[0]
    nw = len(ui)
    W = max(P, ((nw + P - 1) // P) * P)

    bwin = np.zeros(W, dtype=np.float32)
    cwin = np.zeros(W, dtype=np.float32)  # zero weight => padding contributes 0
    bwin[:nw] = be[ui].astype(np.float32)
    cwin[:nw] = ce[ui].astype(np.float32)

    bw_arr = np.ascontiguousarray(np.broadcast_to(bwin, (P, W)))
    cw_arr = np.ascontiguousarray(np.broadcast_to(cwin, (P, W)))
    pq_arr = np.ascontiguousarray(
        np.broadcast_to(np.array([p_fold, q_fold], dtype=np.float32), (P, 2))
    )

    nc = _get_module(W)

    from concourse.bass_utils import run_bass_kernel_spmd

    kf = k[:, 0]
    in_maps = []
    for core in range(N_CORES):
        shard = kf[core * SHARD : (core + 1) * SHARD]
        kk = np.ascontiguousarray(shard.reshape(GROUPS, P).T)  # [P, GROUPS]
        in_maps.append({"kk": kk, "bw": bw_arr, "cw": cw_arr, "pq": pq_arr})

    global _LAST_IN_MAPS
    _LAST_IN_MAPS = in_maps
    results = run_bass_kernel_spmd(nc, in_maps, core_ids=list(range(N_CORES)))
    out = np.concatenate(
        [r["out"].T.reshape(-1) for r in results.results]
    )  # [P,G] -> [G*P] per core
    return out.astype(np.float32)
